# revision 36
# baseline (speedup 1.0000x reference)
"""MiniGPT (dense transformer) Trainium2 Bass kernel — v3.

Sharding: 8 cores = 4 sequences (DP) x sequence-parallel T-split (2).
  core c: seq = c//2, half = c%2 owns tokens [half*512, (half+1)*512).

Each core computes QKV/O-proj/FFN for its 512 tokens with FULL weights
(no tensor-parallel splits -> no AllReduce). Causal attention needs K/V
for all of [0, T): two pairwise AllGathers per layer (K then V, fp16),
issued as soon as each is computed and consumed after the Q projections
so the wire time overlaps compute. Causality is data-driven so both
cores run the same program (SPMD):
  - per-core additive exp-bias column zeroes all-future j-tiles (half 0)
  - per-core 0/1 fp16 mask multiplies the lower-triangular slice of each
    j-tile's exp'd scores (vector engine)

Everything flows fp16 (weights + activations; f32 residual and PSUM):
fp16 matmuls run 1 cyc/row at any N and enable Fast Weight Load, unlike
f32r which disables FWL. LN rstd uses a bit-trick rsqrt + 2 Newton steps
on the vector engine (no scalar-engine act-table switches; the scalar
engine stays on the exp table). w1/w2/wlm and the logits output use
pre-tiled DRAM layouts so every DMA is a contiguous per-partition run.

Layouts per core:
  residual x: SBUF [128, 4, 768] f32, token-major (part=t%128, chunk t//128)
  hT:         [128, 6, 512] fp16 (part=d%128, kt=d//128) via PE transposes
  qT/kT:      [128, (slot,) 6pair, 512] fp16, head pair-packed
              (part = 64*(h%2)+hd); kT_sb is AllGather-slot-major
  v:          [128, 2slot*4, 12h, 65] fp16, token-major + ones column
              (softmax denominator falls out of PV as partition 64); the
              ones column travels through the AllGather
  scores^T:   PSUM [128 j, 1024] per (pair, jt) — both heads side by side,
              written by two row-tiled K=64 matmuls (PE tiles (0,0)/(64,0))
  attn out:   po [65, 512] -> normalize via double transpose -> oTall
              [128, 6, 512] (part = attn-dim, pair-stacked) -> K=128 O-proj
  lm_head:    wlm fp16 pre-tiled [63, 128, 3072]; logits emitted fp16 into
              a tiled [63, 4, 128, 512] output the host re-assembles
"""

import sys
import numpy as np

for _p in ("/opt/trn_rl_repo",):
    if _p not in sys.path:
        sys.path.insert(0, _p)

import concourse.bass as bass
import concourse.tile as tile
from concourse import bacc, mybir
from concourse import bass_utils
from concourse.masks import make_identity
from contextlib import ExitStack

F32 = mybir.dt.float32
F16 = mybir.dt.float16
F8 = mybir.dt.float8e4
I32 = mybir.dt.int32
AF = mybir.ActivationFunctionType
ALU = mybir.AluOpType

V, D, H, L, T, B = 32000, 768, 12, 4, 1024, 4
HD = D // H            # 64
NCORES = 8
TL = T // 2            # 512 local tokens
P = 128
NCT = TL // P          # 4 local token chunks
NJ = T // P            # 8 key chunks
NPAIR = H // 2         # 6 head pairs
KD = D // P            # 6 contraction chunks
F1 = 4 * D             # 3072
MQ = F1 // P           # 24 ffn chunks
KW = NPAIR * TL                    # 3072 cols: k shard (fp8)
VW = NCT * H * (HD + 1)            # 3120 fp16 cols: v shard (with ones)
NVT = (V + 511) // 512             # 63 lm_head vocab tiles (last padded)
RG = [[0, 1], [2, 3], [4, 5], [6, 7]]
NEG = -60000.0


def build_program(bias_flags):
    nc = bacc.Bacc(
        "TRN2",
        target_bir_lowering=False,
        debug=False,
        enable_asserts=False,
        num_devices=NCORES,
    )

    d = {}
    d["x0"] = nc.dram_tensor("x0", [TL, D], F32, kind="ExternalInput").ap()
    d["wqkv"] = nc.dram_tensor("wqkv", [L, D, 3 * D], F16, kind="ExternalInput").ap()
    d["wo"] = nc.dram_tensor("wo", [L, D, D], F16, kind="ExternalInput").ap()
    d["w1t"] = nc.dram_tensor("w1t", [L, KD, P, 3072], F16, kind="ExternalInput").ap()
    d["w2t"] = nc.dram_tensor("w2t", [L, KD, P, 3072], F16, kind="ExternalInput").ap()
    d["wlmt"] = nc.dram_tensor("wlmt", [NVT, P, 3072], F16, kind="ExternalInput").ap()
    d["mask"] = nc.dram_tensor("mask", [P, NJ, TL], F16, kind="ExternalInput").ap()
    d["ebias"] = nc.dram_tensor("ebias", [P, NJ], F32, kind="ExternalInput").ap()
    d["bqk"] = nc.dram_tensor("bqk", [L, P, 12], F32, kind="ExternalInput").ap()
    d["bv"] = nc.dram_tensor("bv", [L, D], F16, kind="ExternalInput").ap()
    d["bo"] = nc.dram_tensor("bo", [L, D], F16, kind="ExternalInput").ap()
    d["b1"] = nc.dram_tensor("b1", [L, P, MQ], F32, kind="ExternalInput").ap()
    d["b2"] = nc.dram_tensor("b2", [L, D], F16, kind="ExternalInput").ap()
    d["blm"] = nc.dram_tensor("blm", [NVT * 512], F16, kind="ExternalInput").ap()
    d["out"] = nc.dram_tensor(
        "logits", [NVT, P, NCT, 512], F16, kind="ExternalOutput").ap()

    with tile.TileContext(nc) as tc, ExitStack() as ctx:
        _body(ctx, tc, bias_flags, d)
    nc.compile()
    return nc


def _body(ctx, tc, bf, d):
    nc = tc.nc
    pool = lambda name, bufs, **kw: ctx.enter_context(
        tc.tile_pool(name=name, bufs=bufs, **kw))

    const = pool("const", 1)
    ln_p = pool("ln", 4)
    lnh_p = pool("lnh", 4)
    x_p = pool("x", 1)
    hT_p = pool("hT", 2)
    dram = pool("dram", 2, space="DRAM")

    lctx = ctx.enter_context(ExitStack())
    lpool = lambda name, bufs, **kw: lctx.enter_context(
        tc.tile_pool(name=name, bufs=bufs, **kw))
    wqkv_p = lpool("wqkv", 1)
    wo_p = lpool("wo", 1)
    w1_p = lpool("w1", 3)
    w2_p = lpool("w2", 2)
    qT_p = lpool("qT", 1)
    kTl_p = lpool("kTl", 1)
    vl_p = lpool("vl", 1)
    kT_p = lpool("kT", 1)
    v_p = lpool("v", 1)
    es_p = lpool("es", 2)
    oT_p = lpool("oT", 4)
    on_p = lpool("on", 2)
    oTa_p = lpool("oTa", 1)
    um_p = lpool("um", 2)
    bias_p = lpool("bias", 2)

    ps_p = pool("ps", 4, space="PSUM")       # 1-bank tiles
    psy_p = pool("psy", 2, space="PSUM")     # [128,1024] 2-bank tiles

    # constants / per-core data
    ident = const.tile([P, P], F16)
    make_identity(nc, ident)
    mask_sb = const.tile([P, NJ, TL], F16)
    nc.sync.dma_start(mask_sb, d["mask"])
    ebias_sb = const.tile([P, NJ], F32)
    nc.sync.dma_start(ebias_sb, d["ebias"])
    ones_row = None
    if any(bf.values()):
        ones_row = const.tile([1, P], F16)
        nc.vector.memset(ones_row, 1.0)

    # residual
    x_sb = x_p.tile([P, NCT, D], F32)
    nc.sync.dma_start(x_sb, d["x0"].rearrange("(n p) t -> p n t", p=P))

    def bias_mm(psum_ap, brow_ap):
        nc.tensor.matmul(psum_ap, ones_row, brow_ap, start=False, stop=False)

    def ln_scalar_chunk(tcl):
        """Stats + rsqrt (vector engine bit-trick) + normalized h fp16."""
        xc = x_sb[:, tcl, :]
        st = ln_p.tile([P, 3, 6], F32, tag="st")
        for s in range(3):
            nc.vector.bn_stats(st[:, s, :], xc[:, s * 256:(s + 1) * 256])
        mv = ln_p.tile([P, 2], F32, tag="mv")
        nc.vector.bn_aggr(mv, st)
        w = ln_p.tile([P, 1], F32, tag="w")
        nc.vector.tensor_scalar_add(w, mv[:, 1:2], 1e-5)
        y = ln_p.tile([P, 1], F32, tag="y")
        yi = y.bitcast(I32)
        nc.vector.tensor_scalar(
            yi, w.bitcast(I32), 1, -1,
            op0=ALU.logical_shift_right, op1=ALU.bitwise_xor)
        nc.vector.tensor_scalar_add(yi, yi, 0x5F3759E0)
        t1 = ln_p.tile([P, 1], F32, tag="t1")
        for _ in range(2):
            nc.vector.tensor_mul(t1, y, y)
            nc.vector.tensor_mul(t1, t1, w)
            nc.vector.tensor_scalar(t1, t1, -0.5, 1.5, op0=ALU.mult, op1=ALU.add)
            nc.vector.tensor_mul(y, y, t1)
        nm = ln_p.tile([P, 1], F32, tag="nm")
        nc.vector.scalar_tensor_tensor(
            nm, mv[:, 0:1], -1.0, y, op0=ALU.mult, op1=ALU.mult)
        h = lnh_p.tile([P, D], F16, tag="h")
        nc.scalar.activation(h, xc, AF.Identity, bias=nm, scale=y)
        return h

    def ln_tp_chunk(hT, tcl, h):
        for kt in range(KD):
            pt = ps_p.tile([P, P], F16, tag="ps")
            nc.tensor.transpose(pt, h[:, kt * P:(kt + 1) * P], ident)
            nc.vector.tensor_copy(hT[:, kt, tcl * P:(tcl + 1) * P], pt)

    # per-layer state carried between phases
    hT_cur = {}
    kv_bufs = {}
    wsb = {}

    def load_layer_weights(l):
        wqkv_sb = wqkv_p.tile([P, KD, 3 * D], F16, tag="wqkv")
        nc.sync.dma_start(wqkv_sb, d["wqkv"][l].rearrange("(k p) n -> p k n", p=P))
        wo_sb = wo_p.tile([P, KD, D], F16, tag="wo")
        nc.sync.dma_start(wo_sb, d["wo"][l].rearrange("(k p) n -> p k n", p=P))
        bqk_sb = brow_v = brow_o = brow_2 = b1_sb = None
        if bf["qk"]:
            bqk_sb = bias_p.tile([P, 12], F32, tag="bqk")
            nc.sync.dma_start(bqk_sb, d["bqk"][l])
        if bf["v"]:
            brow_v = bias_p.tile([1, D], F16, tag="bv")
            nc.sync.dma_start(brow_v, d["bv"][l][None, :])
        if bf["o"]:
            brow_o = bias_p.tile([1, D], F16, tag="bo")
            nc.sync.dma_start(brow_o, d["bo"][l][None, :])
        if bf["b1"]:
            b1_sb = bias_p.tile([P, MQ], F32, tag="b1")
            nc.sync.dma_start(b1_sb, d["b1"][l])
        if bf["b2"]:
            brow_2 = bias_p.tile([1, D], F16, tag="b2")
            nc.sync.dma_start(brow_2, d["b2"][l][None, :])
        wsb[l] = (wqkv_sb, wo_sb, bqk_sb, brow_v, brow_o, brow_2, b1_sb)

    def kv_ln(l, part):
        """LN1 scalar work for token half `part` (no PE instructions)."""
        if part == 0:
            hT_cur[l] = hT_p.tile([P, KD, TL], F16, tag="hT", name="hTkv")
            vloc = vl_p.tile([P, NCT, H, HD + 1], F16, tag="vl", name="vloc")
            nc.gpsimd.memset(vloc[:, :, :, HD:HD + 1], 1.0)
            kv_bufs[l] = (
                kTl_p.tile([P, NPAIR, TL], F8, tag="kTl", name="kTloc"),
                vloc,
                dram.tile([P, KW], F8, tag="kin", name="kin"),
                dram.tile([2, P, KW], F8, tag="kout", name="kout"),
                dram.tile([P, VW], F16, tag="vin", name="vin"),
                dram.tile([2, P, VW], F16, tag="vout", name="vout"),
            )
        return [ln_scalar_chunk(tcl) for tcl in (2 * part, 2 * part + 1)]

    def kv_proj(l, part, hs):
        """Transposes + K/V projections for token half `part`; each half is
        published + AllGathered immediately so the wire overlaps compute."""
        wqkv_sb = wsb[l][0]
        bqk_sb, brow_v = wsb[l][2], wsb[l][3]
        hT = hT_cur[l]
        kT_loc, v_loc, k_in, k_out, v_in, v_out = kv_bufs[l]
        tcls = (2 * part, 2 * part + 1)
        for tcl, h in zip(tcls, hs):
            ln_tp_chunk(hT, tcl, h)
        t0 = part * 256
        for pair in range(NPAIR):
            ps = ps_p.tile([P, 256], F32, tag="ps")
            for kt in range(KD):
                nc.tensor.matmul(
                    ps, wqkv_sb[:, kt, D + pair * P:D + (pair + 1) * P],
                    hT[:, kt, t0:t0 + 256],
                    start=(kt == 0), stop=(kt == KD - 1))
            if bf["qk"]:
                nc.scalar.activation(kT_loc[:, pair, t0:t0 + 256], ps, AF.Copy,
                                     bias=bqk_sb[:, 6 + pair:7 + pair])
            else:
                nc.vector.tensor_copy(kT_loc[:, pair, t0:t0 + 256], ps)
        if part == 0:
            # V projections are deferred to part 1, after the K publish, so
            # they overlap the K AllGather wire
            return
        nc.gpsimd.dma_start(
            k_in.rearrange("p (a j) -> p a j", a=NPAIR), kT_loc)
        nc.gpsimd.collective_compute(
            "AllGather", ALU.bypass, replica_groups=RG,
            ins=[k_in.opt()], outs=[k_out.opt()])
        for jcl in range(NCT):
            ps = psy_p.tile([P, 1024], F32, tag="psy")
            for n0, nw in ((0, 512), (512, 256)):
                for kt in range(KD):
                    nc.tensor.matmul(
                        ps[:, n0:n0 + nw],
                        hT[:, kt, jcl * P:(jcl + 1) * P],
                        wqkv_sb[:, kt, 2 * D + n0:2 * D + n0 + nw],
                        start=(kt == 0), stop=(kt == KD - 1))
                if bf["v"]:
                    bias_mm(ps[:, n0:n0 + nw], brow_v[:, n0:n0 + nw])
            nc.vector.tensor_copy(
                v_loc[:, jcl, :, 0:HD],
                ps[:, 0:D].rearrange("p (h e) -> p h e", h=H))
        nc.gpsimd.dma_start(
            v_in.rearrange("p (c h e) -> p c h e", c=NCT, h=H), v_loc)
        nc.gpsimd.collective_compute(
            "AllGather", ALU.bypass, replica_groups=RG,
            ins=[v_in.opt()], outs=[v_out.opt()])

    def attn_body(l):
        wqkv_sb, wo_sb, bqk_sb, _, brow_o, _, _ = wsb[l]
        hT = hT_cur[l]
        _, _, _, k_out, _, v_out = kv_bufs[l]
        # Q projections (overlap the in-flight AllGathers)
        qT = qT_p.tile([P, NPAIR, TL], F16, tag="qT")
        for pair in range(NPAIR):
            ps = ps_p.tile([P, TL], F32, tag="ps")
            for kt in range(KD):
                nc.tensor.matmul(
                    ps, wqkv_sb[:, kt, pair * P:(pair + 1) * P], hT[:, kt, :],
                    start=(kt == 0), stop=(kt == KD - 1))
            if bf["qk"]:
                nc.scalar.activation(qT[:, pair, :], ps, AF.Copy,
                                     bias=bqk_sb[:, pair:pair + 1])
            else:
                nc.vector.tensor_copy(qT[:, pair, :], ps)
        # K/V readback (waits on each AllGather via tile deps)
        kT_sb = kT_p.tile([P, 2, NPAIR, TL], F8, tag="kT")
        v_sb = v_p.tile([P, 2 * NCT, H, HD + 1], F16, tag="v")
        for s in range(2):
            nc.gpsimd.dma_start(
                kT_sb[:, s, :, :],
                k_out[s].rearrange("p (a j) -> p a j", a=NPAIR))
            nc.gpsimd.dma_start(
                v_sb[:, s * NCT:(s + 1) * NCT, :, :],
                v_out[s].rearrange("p (c h e) -> p c h e", c=NCT, h=H))
        # attention per head pair
        oTall = oTa_p.tile([P, NPAIR, TL], F16, tag="oTa")
        for pair in range(NPAIR):
            es = es_p.tile([P, NJ, 2 * TL], F16, tag="es")
            for jt in range(NJ):
                psc = psy_p.tile([P, 1024], F32, tag="psy")
                for hh in range(2):
                    nc.tensor.matmul(
                        psc[:, hh * TL:(hh + 1) * TL],
                        kT_sb[hh * HD:(hh + 1) * HD, jt // NCT, pair,
                              (jt % NCT) * P:(jt % NCT + 1) * P],
                        qT[hh * HD:(hh + 1) * HD, pair, :],
                        start=True, stop=True)
                nc.scalar.activation(es[:, jt, :], psc, AF.Exp,
                                     scale=0.125, bias=ebias_sb[:, jt:jt + 1])
                w = (jt % NCT + 1) * P
                for hh in range(2):
                    nc.vector.tensor_mul(
                        es[:, jt, hh * TL:hh * TL + w],
                        es[:, jt, hh * TL:hh * TL + w],
                        mask_sb[:, jt, 0:w])
            oT = [None, None]
            for hh in range(2):
                po = ps_p.tile([HD + 1, TL], F32, tag="ps")
                for jt in range(NJ):
                    nc.tensor.matmul(
                        po, v_sb[:, jt, 2 * pair + hh, :],
                        es[:, jt, hh * TL:(hh + 1) * TL],
                        start=(jt == 0), stop=(jt == NJ - 1))
                oT[hh] = oT_p.tile([HD + 1, TL], F16, tag="oT", name="oT")
                nc.vector.tensor_copy(oT[hh], po)
            for tcl in range(NCT):
                on2 = on_p.tile([P, P], F16, tag="on")
                for hh in range(2):
                    pt = ps_p.tile([P, P], F16, tag="ps")
                    nc.tensor.transpose(
                        pt[:, 0:HD + 1], oT[hh][:, tcl * P:(tcl + 1) * P],
                        ident[0:HD + 1, 0:HD + 1])
                    rc = ln_p.tile([P, 1], F32, tag="rc")
                    nc.vector.reciprocal(rc, pt[:, HD:HD + 1])
                    nc.vector.tensor_scalar(
                        on2[:, hh * HD:(hh + 1) * HD], pt[:, 0:HD], rc, None,
                        op0=ALU.mult)
                pt2 = ps_p.tile([P, P], F16, tag="ps")
                nc.tensor.transpose(pt2, on2, ident)
                if pair % 2 == 0:
                    nc.scalar.activation(
                        oTall[:, pair, tcl * P:(tcl + 1) * P], pt2, AF.Copy)
                else:
                    nc.vector.tensor_copy(
                        oTall[:, pair, tcl * P:(tcl + 1) * P], pt2)
        # O-projection + residual add + LN2 scalar work per chunk (so the
        # LN chain latency hides behind the remaining O-proj matmuls)
        hs2 = []
        for tcl in range(NCT):
            py = psy_p.tile([P, 1024], F32, tag="psy")
            for n0, nw in ((0, 512), (512, 256)):
                for c in range(NPAIR):
                    nc.tensor.matmul(
                        py[:, n0:n0 + nw],
                        oTall[:, c, tcl * P:(tcl + 1) * P],
                        wo_sb[:, c, n0:n0 + nw],
                        start=(c == 0), stop=(c == NPAIR - 1))
                if bf["o"]:
                    bias_mm(py[:, n0:n0 + nw], brow_o[:, n0:n0 + nw])
            nc.vector.tensor_add(x_sb[:, tcl, :], x_sb[:, tcl, :], py[:, 0:D])
            hs2.append(ln_scalar_chunk(tcl))
        hT2 = hT_p.tile([P, KD, TL], F16, tag="hT")
        for tcl in range(NCT):
            ln_tp_chunk(hT2, tcl, hs2[tcl])
        return hT2

    def ffn_quarter(l, hT2, q, ln_after=None):
        """FFN for token quarter q; if ln_after, emit kv_ln(l+1, part) right
        after the residual adds so its vector work overlaps the next PE
        stretch."""
        brow_2, b1_sb = wsb[l][5], wsb[l][6]
        pys = [psy_p.tile([P, 1024], F32, tag="psy", name="pys")
               for _ in range(2)]
        for m in range(MQ):
            mg, mi = m // 4, m % 4
            if mi == 0:
                w1g = w1_p.tile([P, KD, 512], F16, tag="w1")
                nc.sync.dma_start(
                    w1g, d["w1t"][l, mg].rearrange("p (k f) -> p k f", k=KD))
                w2g = w2_p.tile([P, 4, D], F16, tag="w2")
                nc.sync.dma_start(
                    w2g, d["w2t"][l, mg].rearrange("p (g n) -> p g n", g=4))
            pu = ps_p.tile([P, 256], F32, tag="ps")
            for kt in range(KD):
                nc.tensor.matmul(
                    pu, w1g[:, kt, mi * P:(mi + 1) * P],
                    hT2[:, kt, q * 256:(q + 1) * 256],
                    start=(kt == 0), stop=(kt == KD - 1))
            um = um_p.tile([P, 256], F16, tag="um")
            if bf["b1"]:
                nc.scalar.activation(um, pu, AF.Relu, bias=b1_sb[:, m:m + 1])
            else:
                nc.scalar.activation(um, pu, AF.Relu)
            for t2 in range(2):
                for n0, nw in ((0, 512), (512, 256)):
                    nc.tensor.matmul(
                        pys[t2][:, n0:n0 + nw],
                        um[:, t2 * P:(t2 + 1) * P], w2g[:, mi, n0:n0 + nw],
                        start=(m == 0), stop=(m == MQ - 1))
        for t2 in range(2):
            if bf["b2"]:
                for n0, nw in ((0, 512), (512, 256)):
                    bias_mm(pys[t2][:, n0:n0 + nw], brow_2[:, n0:n0 + nw])
            tcl = q * 2 + t2
            nc.vector.tensor_add(x_sb[:, tcl, :], x_sb[:, tcl, :],
                                 pys[t2][:, 0:D])
        if ln_after is not None:
            return kv_ln(*ln_after)
        return None

    # ---------------- schedule ----------------
    load_layer_weights(0)
    hs = kv_ln(0, 0)
    kv_proj(0, 0, hs)
    hs = kv_ln(0, 1)
    kv_proj(0, 1, hs)
    for l in range(L):
        hT2 = attn_body(l)
        if l + 1 < L:
            load_layer_weights(l + 1)
            hs0 = ffn_quarter(l, hT2, 0, ln_after=(l + 1, 0))
            ffn_quarter(l, hT2, 1)
            kv_proj(l + 1, 0, hs0)
            hs1 = kv_ln(l + 1, 1)
            kv_proj(l + 1, 1, hs1)
        else:
            ffn_quarter(l, hT2, 0)
            ffn_quarter(l, hT2, 1)

    # ---------------- final LN + lm_head ----------------
    hfT = hT_p.tile([P, KD, TL], F16, tag="hT")
    for tcl in range(NCT):
        ln_tp_chunk(hfT, tcl, ln_scalar_chunk(tcl))
    lctx.close()
    lmw_p = ctx.enter_context(tc.tile_pool(name="lmw", bufs=3))
    lmo_p = ctx.enter_context(tc.tile_pool(name="lmo", bufs=4))
    brow_lm = None
    if bf["lm"]:
        brow_lm = lmo_p.tile([1, NVT * 512], F16, tag="blm")
        nc.sync.dma_start(brow_lm, d["blm"][None, :])
    for vt in range(NVT):
        wt = lmw_p.tile([P, KD, 512], F16, tag="lmw")
        nc.sync.dma_start(
            wt, d["wlmt"][vt].rearrange("p (k w) -> p k w", k=KD))
        lo = lmo_p.tile([P, NCT, 512], F16, tag="lmo")
        for tcg in range(NCT):
            pl = ps_p.tile([P, 512], F32, tag="ps")
            for kt in range(KD):
                nc.tensor.matmul(
                    pl, hfT[:, kt, tcg * P:(tcg + 1) * P], wt[:, kt, :],
                    start=(kt == 0), stop=(kt == KD - 1))
            if bf["lm"]:
                bias_mm(pl, brow_lm[:, vt * 512:(vt + 1) * 512])
            if tcg % 2 == 0:
                nc.scalar.activation(lo[:, tcg, :], pl, AF.Copy)
            else:
                nc.vector.tensor_copy(lo[:, tcg, :], pl)
        nc.sync.dma_start(d["out"][vt], lo)


# ---------------------------------------------------------------------------
# host side
# ---------------------------------------------------------------------------

_CACHE = {}


def _get_program(bias_flags):
    key = tuple(sorted(bias_flags.items()))
    if key not in _CACHE:
        _CACHE[key] = build_program(bias_flags)
    return _CACHE[key]


def make_in_maps(idx, tok_emb, pos_emb, wq, wk, wv, wo, bo,
                 ln1_g, ln1_b, ln2_g, ln2_b, w1, b1, w2, b2,
                 lnf_g, lnf_b, w_lm, b_lm):
    f = lambda a: np.asarray(a, dtype=np.float32)
    idx = np.asarray(idx)
    tok_emb, pos_emb = f(tok_emb), f(pos_emb)
    wq, wk, wv, wo, bo = f(wq), f(wk), f(wv), f(wo), f(bo)
    ln1_g, ln1_b, ln2_g, ln2_b = f(ln1_g), f(ln1_b), f(ln2_g), f(ln2_b)
    w1, b1, w2, b2 = f(w1), f(b1), f(w2), f(b2)
    lnf_g, lnf_b, w_lm, b_lm = f(lnf_g), f(lnf_b), f(w_lm), f(b_lm)

    # fold LN affine into following matmuls (exact when g=1, b=0)
    wq_f = ln1_g[:, :, None] * wq
    wk_f = ln1_g[:, :, None] * wk
    wv_f = ln1_g[:, :, None] * wv
    bq_f = np.einsum("ld,ldo->lo", ln1_b, wq)
    bk_f = np.einsum("ld,ldo->lo", ln1_b, wk)
    bv_f = np.einsum("ld,ldo->lo", ln1_b, wv)
    w1_f = ln2_g[:, :, None] * w1
    b1_f = b1 + np.einsum("ld,ldo->lo", ln2_b, w1)
    wlm_f = lnf_g[:, None] * w_lm
    blm_f = b_lm + lnf_b @ w_lm

    bias_flags = {
        "qk": bool(np.any(bq_f) or np.any(bk_f)),
        "v": bool(np.any(bv_f)),
        "o": bool(np.any(bo)),
        "b1": bool(np.any(b1_f)),
        "b2": bool(np.any(b2)),
        "lm": bool(np.any(blm_f)),
    }

    h16 = lambda a: np.ascontiguousarray(a, dtype=np.float16)
    wqkv_c = h16(np.concatenate([wq_f, wk_f, wv_f], axis=2))
    wo_c = h16(wo)
    # pre-tiled layouts: one contiguous per-partition run per DMA
    w1t = h16(w1_f.reshape(L, KD, P, 6, 512).transpose(0, 3, 2, 1, 4)
              .reshape(L, KD, P, 3072))
    w2t = h16(w2.reshape(L, 6, 4, P, D).transpose(0, 1, 3, 2, 4)
              .reshape(L, KD, P, 3072))
    wlm_pad = np.zeros((D, NVT * 512), np.float32)
    wlm_pad[:, :V] = wlm_f
    wlmt = h16(wlm_pad.reshape(KD, P, NVT, 512).transpose(2, 1, 0, 3)
               .reshape(NVT, P, 3072))
    blm_pad = np.zeros((NVT * 512,), np.float32)
    blm_pad[:V] = blm_f
    # per-partition bias columns in the pair-packed output layout
    bqk_c = np.ascontiguousarray(np.concatenate(
        [bq_f.reshape(L, 6, P).transpose(0, 2, 1),
         bk_f.reshape(L, 6, P).transpose(0, 2, 1)], axis=2), dtype=np.float32)
    b1_c = np.ascontiguousarray(
        b1_f.reshape(L, MQ, P).transpose(0, 2, 1), dtype=np.float32)

    jg = np.arange(T)
    in_maps = []
    for c in range(NCORES):
        seq, half = c // 2, c % 2
        x0 = tok_emb[idx[seq]] + pos_emb[:T]
        tg = half * TL + np.arange(TL)
        allow = (jg[:, None] <= tg[None, :])             # [T, TL]
        mask = allow.reshape(NJ, P, TL).transpose(1, 0, 2)
        ebias = np.zeros((P, NJ), dtype=np.float32)
        if half == 0:
            ebias[:, NCT:] = NEG
        in_maps.append({
            "x0": np.ascontiguousarray(
                x0[half * TL:(half + 1) * TL], dtype=np.float32),
            "wqkv": wqkv_c, "wo": wo_c, "w1t": w1t, "w2t": w2t, "wlmt": wlmt,
            "mask": np.ascontiguousarray(mask, dtype=np.float16),
            "ebias": ebias,
            "bqk": bqk_c, "bv": h16(bv_f), "bo": h16(bo),
            "b1": b1_c, "b2": h16(b2), "blm": h16(blm_pad),
        })
    return in_maps, bias_flags


def assemble(outs):
    logits = np.empty((B, T, V), dtype=np.float32)
    for seq in range(B):
        for half in range(2):
            o = outs[2 * seq + half]          # [NVT, P, NCT, 512] fp16
            lg = o.transpose(2, 1, 0, 3).reshape(TL, NVT * 512)
            logits[seq, half * TL:(half + 1) * TL] = \
                lg[:, :V].astype(np.float32)
    return logits


def kernel(**inputs):
    in_maps, bias_flags = make_in_maps(**inputs)
    nc = _get_program(bias_flags)
    res = bass_utils.run_bass_kernel_spmd(
        nc, in_maps, core_ids=list(range(NCORES)))
    return assemble([res.results[c]["logits"] for c in range(NCORES)])
